# revision 1
# baseline (speedup 1.0000x reference)
"""Trainium2 Bass kernel for nn_DenseAttention (sparse_attention, C=31, B=D=1024).

Strategy (class-parallel over 8 NeuronCores):
- Each core handles 4 classes (core 7: 3 real + 1 zero dummy).
- Per class on device: xBT = K^T-weighted matmul of x, computed as an fp16
  leading term (Kh16 @ xh16, fp32 PSUM) plus ONE fp8e4 DoubleRow correction
  matmul per d-chunk that carries BOTH cross products at 2x PE rate:
  slot0 = fp8(K) @ fp8((x - xh16)*S), slot1 = fp8((K - Kh16)*S) @ fp8(x).
  Same scheme for the xBBx logits on the allowed cross-domain half: lead
  xbh16 @ xbh16 plus one DoubleRow correction with slots
  (xbl8_i @ xb8_j) + (xb8_i @ xbl8_j), where xbl8 = fp8((xB - xbh16)*S),
  xb8 = fp8(xB). The fp8 m2 operand tile stores lhsT slot order for b<512
  (i windows) and rhs slot order for b>=512 (j windows) - disjoint ranges.
- Then label-equality masking, E = exp(logits - 200), row sums and
  boundary-split partial row sums, AE = sum_c E_c (upper cross block only;
  host mirrors the lower block).
- The reference's softmax is a raw reshape [B,B,C] -> [C, B*B]: softmax groups
  are 31 chunks of 2^20 flat elements crossing class boundaries. Group
  membership of (p=i*B+j, c) is (31p+c)>>20; per class each group is a
  contiguous p-range, so group sums are assembled on the host from whole-row
  sums plus lo-part partial sums at the <=30 boundary rows per class.
- exp shift is the constant 200 (any per-group-constant shift cancels in the
  softmax ratio; 200 keeps everything in fp32 range and reproduces the
  reference's masked-element underflow-to-zero behaviour exactly).
- Host: sums s_g in fp64, out = (sum_cores AE) / s_{g0(p)} plus corrections at
  the <=30 flat positions per group whose true group differs from g0(p).
"""

import functools

import ml_dtypes
import numpy as np

import concourse.mybir as mybir
import concourse.tile as tile
from concourse import bacc
from concourse.bass_utils import run_bass_kernel_spmd

C, B, D = 31, 1024, 1024
NCORES = 8
CPAD = 4
MHAT = 200.0
SC = 16384.0
ISC = float(1.0 / SC)
M_FLAT = 1 << 20
F32 = mybir.dt.float32
F16 = mybir.dt.float16
F8 = mybir.dt.float8e4
E4M3 = ml_dtypes.float8_e4m3
DR = mybir.MatmulPerfMode.DoubleRow
EXP = mybir.ActivationFunctionType.Exp
ALU = mybir.AluOpType


def _pc(c, g):
    """First p with (31p + c) >= g * 2^20."""
    return (g * M_FLAT - c + 30) // 31


@functools.lru_cache(maxsize=1)
def _build():
    nc = bacc.Bacc("TRN2", target_bir_lowering=False, debug=False,
                   num_devices=NCORES)
    xth_d = nc.dram_tensor("xth", [128, 8 * 1024], F16, kind="ExternalInput")
    xq1_d = nc.dram_tensor("xq1", [128, 16, 1024], F8, kind="ExternalInput")
    khi_d = nc.dram_tensor("khi", [CPAD, 8, 128, 1024], F16, kind="ExternalInput")
    k8_d = nc.dram_tensor("k8", [CPAD, 8, 128, 16, 128], F8, kind="ExternalInput")
    labi_d = nc.dram_tensor("labi", [128, CPAD * 8], F32, kind="ExternalInput")
    labj_d = nc.dram_tensor("labj", [128, CPAD * 512], F32, kind="ExternalInput")
    bias_d = nc.dram_tensor("biasc", [128, CPAD * 8], F32, kind="ExternalInput")

    # upper cross block only (E is symmetric; host mirrors the lower block).
    # Slice-major layouts keep each DMA contiguous in DRAM so the runtime
    # shards it across all 16 DMA engines (strided writes go single-engine).
    oute_d = nc.dram_tensor("out_e", [CPAD * 8, 128, 256], F32,
                            kind="ExternalOutput")

    with tile.TileContext(nc) as tc:
        with (
            tc.tile_pool(name="persist", bufs=1) as pp,
            tc.tile_pool(name="kpool", bufs=2) as kp,
            tc.tile_pool(name="work", bufs=3) as wp,
            tc.tile_pool(name="eqpool", bufs=8) as ep,
            tc.tile_pool(name="psum", bufs=3, space="PSUM") as ps,
        ):
            xth_t = pp.tile([128, 8 * 1024], F16)
            xq1_t = pp.tile([128, 16, 1024], F8)
            xbh_t = pp.tile([128, 8 * 1024], F16)
            xq2_t = pp.tile([128, 16, 1024], F8)
            labi_t = pp.tile([128, CPAD * 8], F32)
            labj_t = pp.tile([128, CPAD * 512], F32)
            bias_t = pp.tile([128, CPAD * 8], F32)
            b200_t = pp.tile([128, 1], F32)

            # class-0/et-0 weights first, then x chunks in consumption order,
            # so the first matmuls start as early as possible
            kh0_t = kp.tile([128, 1024], F16, tag="kh")
            k80_t = kp.tile([128, 16, 128], F8, tag="k8")
            nc.sync.dma_start(out=kh0_t[:], in_=khi_d[0, 0])
            nc.sync.dma_start(out=xth_t[:, 0:1024], in_=xth_d[:, 0:1024])
            nc.sync.dma_start(out=k80_t[:], in_=k8_d[0, 0])
            for dc in range(1, 8):
                nc.sync.dma_start(out=xth_t[:, dc * 1024:(dc + 1) * 1024],
                                  in_=xth_d[:, dc * 1024:(dc + 1) * 1024])
            nc.sync.dma_start(out=bias_t[:], in_=bias_d[:])
            for qc in range(4):
                nc.sync.dma_start(out=xq1_t[:, 4 * qc:4 * qc + 4, :],
                                  in_=xq1_d[:, 4 * qc:4 * qc + 4, :])
            nc.vector.memset(b200_t[:], -MHAT)

            for cl in range(CPAD):
                # ---- matmul1: xBT[e, i] = sum_d K[d,e] * xT[d,i] (+bias) ----
                for et in range(8):
                    if cl == 0 and et == 0:
                        kh_t, k8_t = kh0_t, k80_t
                    else:
                        kh_t = kp.tile([128, 1024], F16, tag="kh")
                        k8_t = kp.tile([128, 16, 128], F8, tag="k8")
                        nc.gpsimd.dma_start(out=kh_t[:], in_=khi_d[cl, et])
                        nc.gpsimd.dma_start(out=k8_t[:], in_=k8_d[cl, et])
                    p1a = ps.tile([128, 512], F32, tag="p1")
                    p1b = ps.tile([128, 512], F32, tag="p1")
                    p2a = ps.tile([128, 512], F32, tag="p2")
                    p2b = ps.tile([128, 512], F32, tag="p2")
                    p1s = [p1a, p1b]
                    p2s = [p2a, p2b]
                    for dc in range(8):
                        w = kh_t[:, dc * 128:(dc + 1) * 128]
                        for ih in range(2):
                            nc.tensor.matmul(
                                out=p1s[ih][:], lhsT=w,
                                rhs=xth_t[:, dc * 1024 + ih * 512:
                                          dc * 1024 + ih * 512 + 512],
                                start=(dc == 0), stop=(dc == 7))
                    for dc in range(8):
                        w8 = k8_t[:, 2 * dc:2 * dc + 2, :]
                        for ih in range(2):
                            nc.tensor.matmul(
                                out=p2s[ih][:], lhsT=w8,
                                rhs=xq1_t[:, 2 * dc:2 * dc + 2,
                                          ih * 512:ih * 512 + 512],
                                start=(dc == 0), stop=(dc == 7),
                                perf_mode=DR)
                    for ih in range(2):
                        p1 = p1s[ih]
                        p2 = p2s[ih]
                        vtmp = wp.tile([128, 512], F32, tag="vtmp")
                        vfull = wp.tile([128, 512], F32, tag="vfull")
                        dtmp = wp.tile([128, 512], F32, tag="dtmp")
                        # p2 evac on ACT (DVE may read only one PSUM input)
                        nc.scalar.activation(
                            out=vtmp[:], in_=p2[:],
                            func=mybir.ActivationFunctionType.Copy, scale=ISC)
                        nc.vector.scalar_tensor_tensor(
                            out=vfull[:], in0=p1[:],
                            scalar=bias_t[:, cl * 8 + et:cl * 8 + et + 1],
                            in1=vtmp[:], op0=ALU.add, op1=ALU.add)
                        osl = slice(et * 1024 + ih * 512, et * 1024 + ih * 512 + 512)
                        nc.scalar.copy(out=xbh_t[:, osl], in_=vfull[:])
                        # m2 fp8 operands: b<512 windows feed lhsT (slot order
                        # xbl8, xb8), b>=512 feed rhs (slot order xb8, xbl8)
                        s_l, s_f = (0, 1) if ih == 0 else (1, 0)
                        wsl = slice(ih * 512, ih * 512 + 512)
                        nc.scalar.copy(out=xq2_t[:, 2 * et + s_f, wsl],
                                       in_=vfull[:])
                        nc.vector.scalar_tensor_tensor(
                            out=dtmp[:], in0=xbh_t[:, osl], scalar=-1.0,
                            in1=vfull[:], op0=ALU.mult, op1=ALU.add)
                        nc.scalar.activation(
                            out=xq2_t[:, 2 * et + s_l, wsl], in_=dtmp[:],
                            func=mybir.ActivationFunctionType.Copy, scale=SC)

                if cl == 0:
                    # M2-only inputs: issued here so the preamble DMA queue
                    # holds only what the first matmuls need
                    nc.sync.dma_start(out=labi_t[:], in_=labi_d[:])
                    nc.sync.dma_start(out=labj_t[:], in_=labj_d[:])

                # label-equality masks for all 4 i-blocks, off the critical
                # m2 evac chain (overlaps m1 of the next et/class)
                eqts = []
                for it in range(4):
                    eqt = ep.tile([128, 512], F32, tag="eqt")
                    nc.vector.tensor_scalar(
                        out=eqt[:], in0=labj_t[:, cl * 512:cl * 512 + 512],
                        scalar1=labi_t[:, cl * 8 + it:cl * 8 + it + 1],
                        scalar2=None, op0=ALU.is_equal)
                    eqts.append(eqt)

                # ---- matmul2 + mask + exp + sums, upper cross block only ----
                for it in range(4):
                    q1a = ps.tile([128, 256], F32, tag="p1")
                    q1b = ps.tile([128, 256], F32, tag="p1")
                    q1h = [q1a, q1b]
                    q2 = ps.tile([128, 512], F32, tag="p2")
                    # corr chain first: its ACT-engine evac overlaps the lead
                    for ec in range(8):
                        nc.tensor.matmul(
                            out=q2[:], lhsT=xq2_t[:, 2 * ec:2 * ec + 2,
                                                  it * 128:it * 128 + 128],
                            rhs=xq2_t[:, 2 * ec:2 * ec + 2, 512:1024],
                            start=(ec == 0), stop=(ec == 7),
                            perf_mode=DR)
                    # lead in two half-width chains: the left half's evac/exp/
                    # DMA overlaps the right half's matmuls, halving the
                    # exposed chain after the very last matmul of the kernel
                    for h in range(2):
                        for ec in range(8):
                            ioff = ec * 1024 + it * 128
                            nc.tensor.matmul(
                                out=q1h[h][:], lhsT=xbh_t[:, ioff:ioff + 128],
                                rhs=xbh_t[:, ec * 1024 + 512 + h * 256:
                                          ec * 1024 + 768 + h * 256],
                                start=(ec == 0), stop=(ec == 7))
                    vtmp = wp.tile([128, 512], F32, tag="vtmp")
                    vfull = wp.tile([128, 512], F32, tag="vfull")
                    mt = wp.tile([128, 512], F32, tag="mt")
                    ext = wp.tile([128, 512], F32, tag="ext")
                    nc.scalar.activation(
                        out=vtmp[:], in_=q2[:],
                        func=mybir.ActivationFunctionType.Copy, scale=ISC)
                    for h in range(2):
                        hs = slice(h * 256, h * 256 + 256)
                        nc.vector.tensor_tensor(
                            out=vfull[:, hs], in0=q1h[h][:], in1=vtmp[:, hs],
                            op=ALU.add)
                        nc.vector.tensor_tensor(
                            out=mt[:, hs], in0=vfull[:, hs],
                            in1=eqts[it][:, hs], op=ALU.mult)
                        nc.scalar.activation(
                            out=ext[:, hs], in_=mt[:, hs], func=EXP,
                            bias=b200_t[:], scale=1.0)
                        # ship E to host (AE sum and lower-row sums are
                        # assembled there in fp64 from these blocks)
                        nc.sync.dma_start(out=oute_d[(cl * 4 + it) * 2 + h],
                                          in_=ext[:, hs])


    nc.compile()
    return nc


def _core_classes():
    return [list(range(c * 4, min(c * 4 + 4, C))) for c in range(NCORES)]


def _thresholds(c):
    """Per-row j-split T[i] for global class c (0 = no boundary in row)."""
    T = np.zeros(B, np.int64)
    for g in range(1, C):
        p = _pc(c, g)
        i0, t = divmod(p, B)
        if t != 0:
            T[i0] = t
    return T


def _prep_inputs(x, labels, kernel, bias):
    xT = np.ascontiguousarray(x.T).astype(np.float64)
    xh16 = xT.astype(np.float16)
    xl8 = ((xT - xh16.astype(np.float64)) * SC).astype(E4M3)
    x8 = xT.astype(np.float32).astype(E4M3)
    xth = np.ascontiguousarray(
        xh16.reshape(8, 128, 1024).transpose(1, 0, 2)).reshape(128, 8 * 1024)
    xq1 = np.empty((128, 16, 1024), E4M3)
    xq1[:, 0::2, :] = xl8.reshape(8, 128, 1024).transpose(1, 0, 2)
    xq1[:, 1::2, :] = x8.reshape(8, 128, 1024).transpose(1, 0, 2)
    in_maps = []
    for classes in _core_classes():
        k4 = np.zeros((CPAD, D, D), np.float32)
        b4 = np.zeros((CPAD, D), np.float32)
        l4 = np.zeros((B, CPAD), np.int32)
        for cl, c in enumerate(classes):
            k4[cl] = kernel[c]
            b4[cl] = bias[c]
            l4[:, cl] = labels[:, c]
        khi = k4.astype(np.float16)
        k8f = k4.astype(E4M3)
        kl8 = ((k4.astype(np.float64) - khi.astype(np.float64)) * SC).astype(E4M3)

        # [cl, d, e] -> [cl, et(8), p(128), dc(8), e(128)]
        def re(a):
            a = a.reshape(CPAD, 8, 128, 8, 128)          # cl, dc, p, et, e
            return np.ascontiguousarray(a.transpose(0, 3, 2, 1, 4))

        khi_r = re(khi).reshape(CPAD, 8, 128, 1024)
        k8 = np.empty((CPAD, 8, 128, 16, 128), E4M3)
        k8[..., 0::2, :] = re(k8f)
        k8[..., 1::2, :] = re(kl8)
        labi = l4.reshape(8, 128, CPAD).transpose(1, 2, 0)      # p, cl, it
        labi = np.ascontiguousarray(labi.astype(np.float32)).reshape(128, CPAD * 8)
        labj = np.broadcast_to(
            l4[512:, :].T.astype(np.float32)[None, :, :], (128, CPAD, 512)
        ).reshape(128, CPAD * 512).copy()
        biasc = b4.reshape(CPAD, 8, 128).transpose(2, 0, 1)     # p, cl, et
        biasc = np.ascontiguousarray(biasc.astype(np.float32)).reshape(128, CPAD * 8)
        in_maps.append(dict(
            xth=xth, xq1=xq1, khi=khi_r, k8=k8,
            labi=labi, labj=labj, biasc=biasc,
        ))
    return in_maps


def _assemble(results, x, labels, kernel, bias):
    s = np.zeros(C, np.float64)
    AE_tot = np.zeros((B, B), np.float64)
    i_idx = np.arange(B, dtype=np.int64)
    for res, classes in zip(results, _core_classes()):
        # upper cross block [i<512, j>=512]; lower block is its transpose
        ecls = res["out_e"].reshape(CPAD, 4, 2, 128, 256)\
            .transpose(0, 1, 3, 2, 4).reshape(CPAD, 512, 512).astype(np.float64)
        up = ecls.sum(axis=0)
        AE_tot[:512, 512:] += up
        AE_tot[512:, :512] += up.T
        jv = np.arange(512, dtype=np.int64)[:, None]
        ju = np.arange(512, 1024, dtype=np.int64)[None, :]
        for cl, c in enumerate(classes):
            g_row = (31 * (i_idx * B) + c) >> 20
            T = _thresholds(c)
            e_cl = ecls[cl]
            rse = np.concatenate([e_cl.sum(axis=1), e_cl.sum(axis=0)])
            mup = (ju < T[:512][:, None]).astype(np.float64)
            mlow = (jv < T[512:][None, :]).astype(np.float64)
            rslo = np.concatenate([(e_cl * mup).sum(axis=1),
                                   (e_cl * mlow).sum(axis=0)])
            hb = T > 0
            np.add.at(s, g_row[~hb], rse[~hb])
            np.add.at(s, g_row[hb], rslo[hb])
            np.add.at(s, g_row[hb] + 1, (rse[hb] - rslo[hb]))
    p = np.arange(B * B, dtype=np.int64)
    g0 = (31 * p) >> 20
    out = AE_tot * (1.0 / s)[g0].reshape(B, B)

    # corrections at flat positions whose true group g differs from g0(p)
    half = B // 2
    corr = {}  # (i, j) -> list of (c, g)
    for g in range(1, C):
        pB_ = _pc(0, g)
        for c in range(C):
            for pstar in range(_pc(c, g), pB_):
                i, j = divmod(pstar, B)
                cross = (i < half) != (j < half)
                if cross and labels[i, c] == labels[j, c]:
                    corr.setdefault((i, j), []).append((c, g))
    for (i, j), lst in corr.items():
        for c, g in lst:
            vi = x[i].astype(np.float64) @ kernel[c].astype(np.float64) \
                + bias[c].astype(np.float64)
            vj = x[j].astype(np.float64) @ kernel[c].astype(np.float64) \
                + bias[c].astype(np.float64)
            Mij = np.float64(np.float32(vi @ vj))
            E = np.exp(Mij - MHAT)
            out[i, j] += E * (1.0 / s[g] - 1.0 / s[g - 1])
    return out.astype(np.float32)


def _run(inputs, trace=False):
    x = np.asarray(inputs["inputs"], np.float32)
    labels = np.asarray(inputs["labels"])
    kern = np.asarray(inputs["kernel"], np.float32)
    bias = np.asarray(inputs["bias"], np.float32)
    nc = _build()
    in_maps = _prep_inputs(x, labels, kern, bias)
    res = run_bass_kernel_spmd(nc, in_maps, core_ids=list(range(NCORES)),
                               trace=trace)
    out = _assemble(res.results, x, labels, kern, bias)
    return out, res


def kernel(**inputs) -> np.ndarray:
    return _run(inputs, trace=False)[0]

